# revision 21
# baseline (speedup 1.0000x reference)
"""Trainium2 Bass kernel for channel-wise weighted reduction + capped relu.

Computes out[b, s] = capped_relu(sum_c x[b,c,s] * W[c,s] + bias[s]) for
x [64, 256, 4096] f32, W [256, 4096] f32, bias [4096] f32.

Sharding (v2): split the s axis (4096) across 8 NeuronCores — 512 columns
per core, ALL 64 batches. Each core reads x[:, :, s0:s0+512] (32 MiB), its
W slice [256, 512] (0.5 MiB) and bias slice. Compared to batch-parallel
sharding this cuts replicated-weight HBM traffic 8x (36 -> 32.6 MiB/core).

Per-core pipeline (v2):
  - 8 groups of 8 batches. Per group, 2 DMAs of [128c, 8b*512s] (2 MiB
    each, one per channel half) on the sync HWDGE ring.
  - DVE: y = x * Wrep elementwise, one [128, 4096] op per half (Wrep is
    the 0.5 KiB weight half replicated 8x along free at startup).
  - PE: channel reduction via ones[128,1] stationary matmul, one N=512
    chunk per (batch, half), accumulating h0+h1 in PSUM. Optional f32r
    (1 cyc/row vs 4 for f32); products are rounded to f32r by the DVE.
  - ACT drains psum rows to a [1, 4096] staging row; an accumulating
    SWDGE DMA packs it onto out_acc[g*8:(g+1)*8, :] which was preloaded
    with bias (bias add fused for free).
  - Epilogue on [64, 512]: capped relu via mask, single store.
"""

import os

import numpy as np

B, C, S = 64, 256, 4096
NCORES = 8
BPC = B // NCORES          # batches per core (v1 batch sharding)
SPC = S // NCORES          # s-columns per core (v2 s sharding)
H = C // 128               # 2 channel halves

_cache = {}


def _arch():
    return os.environ.get("K_ARCH", "v2")


def _build_nc_v2(fold=2, gfold=1, bufs=4, wbcast=True):
    """S-sharded kernel. Host stages x as [C, B*SPC] (c-major) so every
    DMA chunk is one contiguous run per partition (128 descriptors per
    2 MiB transfer instead of 1024 — HWDGE descriptor-gen was the v2
    bottleneck at 3.6 us per transfer).

    fold:  batches per group whose h0+h1 products are pre-added on DVE
    gfold: batches per group pre-added on GPSIMD (2x DVE cost, idle engine)
    wbcast: read W via a stride-0 broadcast AP instead of materializing
            an 8x-replicated copy (saves 3.5 MiB of SBUF->SBUF DMA).
    """
    import concourse.bacc as bacc
    import concourse.mybir as mybir
    from concourse.tile import TileContext

    f32 = mybir.dt.float32
    Alu = mybir.AluOpType

    NB = B // 8                # 8 groups of 8 batches
    GW = 8 * SPC               # free width of one group tile: 8*512 = 4096

    nc = bacc.Bacc(
        "TRN2",
        target_bir_lowering=False,
        debug=False,
        num_devices=NCORES,
    )

    x_d = nc.dram_tensor("x", [C, B * SPC], f32, kind="ExternalInput").ap()
    w_d = nc.dram_tensor("weights", [C, SPC], f32, kind="ExternalInput").ap()
    b_d = nc.dram_tensor("bias", [SPC], f32, kind="ExternalInput").ap()
    o_d = nc.dram_tensor("out", [B, SPC], f32, kind="ExternalOutput").ap()

    with TileContext(nc) as tc:
        with (
            tc.tile_pool(name="consts", bufs=1) as cpool,
            tc.tile_pool(name="xbuf", bufs=bufs) as xpool,
            tc.tile_pool(name="stg", bufs=2) as spool,
            tc.tile_pool(name="epi", bufs=1) as epool,
            tc.tile_pool(name="ps", bufs=1, space="PSUM") as ppool,
        ):
            WREP = 1 if wbcast else 8
            w_t = [cpool.tile([128, WREP * SPC], f32, name=f"w{h}") for h in range(H)]
            for h in range(H):
                nc.scalar.dma_start(w_t[h][:, 0:SPC], w_d[h * 128:(h + 1) * 128, :])
                if not wbcast:
                    for k in (1, 2, 4):
                        nc.scalar.activation(
                            w_t[h][:, k * SPC:2 * k * SPC], w_t[h][:, 0:k * SPC],
                            mybir.ActivationFunctionType.Copy,
                        )

            ones_t = cpool.tile([128, 1], f32, name="ones_t")
            nc.vector.memset(ones_t[:], 1.0)

            # out_acc preloaded with bias on every row; group sums are
            # packed on with accumulating SWDGE DMAs (fused bias add).
            out_acc = epool.tile([B, SPC], f32, name="out_acc")
            nc.gpsimd.dma_start(out_acc[0:1, :], b_d[None, :])
            k = 1
            while k < B:
                nc.gpsimd.dma_start(out_acc[k:2 * k, :], out_acc[0:k, :])
                k *= 2

            psum_big = ppool.tile([128, GW], f32, name="psum_big")
            rings = [nc.sync, nc.scalar]

            for g in range(NB):
                hb = g % 2              # psum bank half (free-dim half)
                rp = (g // 2) % 2       # psum row pair
                last = g == NB - 1
                xt = [
                    xpool.tile([128, GW], f32, name=f"x_h{h}", tag=f"x{h}", bufs=bufs)
                    for h in range(H)
                ]
                stg = spool.tile([1, GW], f32, name="stg", tag="stg")
                for h in range(H):
                    rings[h].dma_start(
                        xt[h][:, :],
                        x_d[h * 128:(h + 1) * 128, g * GW:(g + 1) * GW],
                    )
                # y = x * W (in place), one [128, 4096] op per half; the
                # last group runs in 2048-wide chunks to shorten the tail.
                nmul = 2 if last else 1
                for mq in range(nmul):
                    ms = slice(mq * GW // nmul, (mq + 1) * GW // nmul)
                    nb_lo = 8 // nmul * mq
                    nb_hi = 8 // nmul * (mq + 1)
                    for h in range(H):
                        if wbcast:
                            wop = (
                                w_t[h][:, 0:SPC]
                                .unsqueeze(1)
                                .broadcast_to([128, (nb_hi - nb_lo), SPC])
                            )
                        else:
                            wop = w_t[h][:, ms]
                        nc.vector.tensor_tensor(
                            xt[h][:, ms].rearrange(
                                "p (b s) -> p b s", b=nb_hi - nb_lo
                            ) if wbcast else xt[h][:, ms],
                            xt[h][:, ms].rearrange(
                                "p (b s) -> p b s", b=nb_hi - nb_lo
                            ) if wbcast else xt[h][:, ms],
                            wop,
                            Alu.mult,
                        )
                    # channel-sum via PE; batch b8 -> row 32*(2*rp + b8//4),
                    # free off 2048*hb + (b8%4)*512 (4 banks per group; bank
                    # half alternates per group so ACT drains don't collide
                    # with the next group's matmul writes). fold/gfold
                    # batches get h0+h1 pre-added on DVE/GPSIMD, halving
                    # their PE stream (f32 moving is 4 cyc/row).
                    for b8 in range(nb_lo, nb_hi):
                        row = 32 * (2 * rp + b8 // 4)
                        off = (GW // 2) * hb + (b8 % 4) * SPC
                        sl = slice(b8 * SPC, (b8 + 1) * SPC)
                        fold_eng = None
                        if b8 >= 8 - fold:
                            fold_eng = nc.vector
                        elif b8 < gfold:
                            fold_eng = nc.gpsimd
                        if fold_eng is not None:
                            fold_eng.tensor_tensor(
                                xt[0][:, sl], xt[0][:, sl], xt[1][:, sl], Alu.add
                            )
                        nh = 1 if fold_eng is not None else H
                        for h in range(nh):
                            nc.tensor.matmul(
                                psum_big[row:row + 1, off:off + SPC],
                                ones_t[:, 0:1],
                                xt[h][:, sl],
                                start=(h == 0),
                                stop=(h == nh - 1),
                                tile_position=(0, row),
                            )
                        if b8 % 4 == 3:
                            # this psum row is complete: drain it now so the
                            # ACT copy overlaps the remaining matmuls
                            half = b8 // 4
                            nc.scalar.activation(
                                stg[:, half * (GW // 2):(half + 1) * (GW // 2)],
                                psum_big[row:row + 1,
                                         (GW // 2) * hb:(GW // 2) * hb + GW // 2],
                                mybir.ActivationFunctionType.Copy,
                            )
                nc.gpsimd.dma_start(
                    out_acc[g * 8:(g + 1) * 8, :],
                    stg[:, :].rearrange("p (b s) -> p b s", b=8),
                    accum_op=Alu.add,
                )

            # capped relu on [64, 512], then single store
            msk = epool.tile([B, SPC], f32, name="msk")
            nc.vector.tensor_scalar(msk[:], out_acc[:, :], 0.0, 1.0, Alu.max, Alu.is_le)
            nc.vector.scalar_tensor_tensor(
                out_acc[:, :], out_acc[:, :], 0.0, msk[:], Alu.max, Alu.mult
            )
            nc.scalar.dma_start(o_d[:, :], out_acc[:, :])

    nc.compile()
    return nc


def _build_nc_v1(fold_batches=2, use_f32r=False):
    """Batch-sharded baseline (8 batches/core, W replicated)."""
    import concourse.bacc as bacc
    import concourse.bass as bass
    import concourse.mybir as mybir
    from concourse.tile import TileContext

    f32 = mybir.dt.float32
    Alu = mybir.AluOpType
    NJ = S // 512
    nc = bacc.Bacc(
        "TRN2",
        target_bir_lowering=False,
        debug=False,
        num_devices=NCORES,
    )

    x_d = nc.dram_tensor("x", [BPC, C, S], f32, kind="ExternalInput").ap()
    w_d = nc.dram_tensor("weights", [C, S], f32, kind="ExternalInput").ap()
    b_d = nc.dram_tensor("bias", [S], f32, kind="ExternalInput").ap()
    o_d = nc.dram_tensor("out", [BPC, S], f32, kind="ExternalOutput").ap()

    with TileContext(nc) as tc:
        NQ = 4
        QS = S // NQ
        with (
            tc.tile_pool(name="consts", bufs=1) as cpool,
            tc.tile_pool(name="xbuf", bufs=3) as xpool,
            tc.tile_pool(name="stg", bufs=2) as spool,
            tc.tile_pool(name="epi", bufs=1) as epool,
            tc.tile_pool(name="ps", bufs=1, space="PSUM") as ppool,
        ):
            w_t = cpool.tile([128, H * S], f32, name="w_t")

            ones_dt = mybir.dt.float32r if use_f32r else f32
            ones_t = cpool.tile([128, 1], ones_dt, name="ones_t")
            nc.vector.memset(ones_t[:], 1.0)

            psum_big = ppool.tile([128, S], f32, name="psum_big")
            out_acc = epool.tile([BPC, S], f32, name="out_acc")
            for bb in range(BPC):
                nc.scalar.dma_start(out_acc[bb:bb + 1, :], b_d[None, :])

            def chunk(base, h, q):
                return slice(base + h * S + q * QS, base + h * S + (q + 1) * QS)

            for b in range(BPC):
                hb = b % 2
                rp = (b // 2) % 2
                xh = [
                    xpool.tile([128, S], f32, name=f"x_h{h}", tag=f"x{h}", bufs=3)
                    for h in range(H)
                ]
                for h in range(H):
                    for dq in range(2):
                        lo, hi = dq * (S // 2), (dq + 1) * (S // 2)
                        if b == 0:
                            nc.sync.dma_start(
                                w_t[:, h * S + lo:h * S + hi],
                                w_d[h * 128:(h + 1) * 128, lo:hi],
                            )
                        nc.sync.dma_start(
                            xh[h][:, lo:hi],
                            x_d[b, h * 128:(h + 1) * 128, lo:hi],
                        )
                fold = b >= BPC - fold_batches
                nhalf = 1 if fold else H
                for q in range(NQ):
                    qs = slice(q * QS, (q + 1) * QS)
                    for h in range(H):
                        out_ap = xh[h][:, qs]
                        if use_f32r:
                            out_ap = out_ap.bitcast(mybir.dt.float32r)
                        nc.vector.tensor_tensor(
                            out_ap, xh[h][:, qs], w_t[:, chunk(0, h, q)],
                            Alu.mult,
                        )
                    if fold:
                        nc.vector.tensor_tensor(
                            xh[0][:, qs], xh[0][:, qs], xh[1][:, qs], Alu.add
                        )
                    for j in (2 * q, 2 * q + 1):
                        row = 32 * (2 * rp + j // 4)
                        off = (S // 2) * hb + (j % 4) * 512
                        for h in range(nhalf):
                            rhs = xh[h][:, j * 512:(j + 1) * 512]
                            lhsT = ones_t[:, 0:1]
                            if use_f32r:
                                rhs = rhs.bitcast(mybir.dt.float32r)
                            nc.tensor.matmul(
                                psum_big[row:row + 1, off:off + 512],
                                lhsT,
                                rhs,
                                start=(h == 0),
                                stop=(h == nhalf - 1),
                                tile_position=(0, row),
                            )
                stg = spool.tile([1, S], f32, name="stg", tag="stg")
                for half in range(2):
                    row = 32 * (2 * rp + half)
                    off = (S // 2) * hb
                    nc.scalar.activation(
                        stg[:, half * (S // 2):(half + 1) * (S // 2)],
                        psum_big[row:row + 1, off:off + S // 2],
                        mybir.ActivationFunctionType.Copy,
                    )
                nc.gpsimd.dma_start(
                    out_acc[b:b + 1, :], stg[:, :], accum_op=Alu.add
                )

            for s0 in (0, S // 2):
                sl = slice(s0, s0 + S // 2)
                msk = epool.tile([BPC, S // 2], f32, name="msk", tag="msk", bufs=1)
                nc.vector.tensor_scalar(msk[:], out_acc[:, sl], 0.0, 1.0, Alu.max, Alu.is_le)
                nc.vector.scalar_tensor_tensor(
                    out_acc[:, sl], out_acc[:, sl], 0.0, msk[:], Alu.max, Alu.mult
                )
                nc.scalar.dma_start(o_d[:, sl], out_acc[:, sl])

    nc.compile()
    return nc


def _build():
    if _arch() == "v1":
        return _build_nc_v1(
            fold_batches=int(os.environ.get("K_FOLD", "2")),
            use_f32r=bool(int(os.environ.get("K_F32R", "0"))),
        )
    return _build_nc_v2(
        fold=int(os.environ.get("K_FOLD", "2")),
        gfold=int(os.environ.get("K_GFOLD", "1")),
        bufs=int(os.environ.get("K_BUFS", "4")),
        wbcast=bool(int(os.environ.get("K_WBCAST", "1"))),
    )


def make_in_maps(x, weights, bias):
    x = np.ascontiguousarray(x, dtype=np.float32)
    weights = np.ascontiguousarray(weights, dtype=np.float32)
    bias = np.ascontiguousarray(bias, dtype=np.float32)
    if _arch() == "v1":
        return [
            {"x": x[i * BPC:(i + 1) * BPC], "weights": weights, "bias": bias}
            for i in range(NCORES)
        ]
    # v2: x staged c-major [C, B*SPC] so each partition's chunk data is one
    # contiguous DRAM run (128 DMA descriptors per 2 MiB transfer).
    return [
        {
            "x": np.ascontiguousarray(
                x[:, :, i * SPC:(i + 1) * SPC].transpose(1, 0, 2)
            ).reshape(C, B * SPC),
            "weights": np.ascontiguousarray(weights[:, i * SPC:(i + 1) * SPC]),
            "bias": np.ascontiguousarray(bias[i * SPC:(i + 1) * SPC]),
        }
        for i in range(NCORES)
    ]


def gather(results):
    if _arch() == "v1":
        return np.concatenate([results[i]["out"] for i in range(NCORES)], axis=0)
    return np.concatenate([results[i]["out"] for i in range(NCORES)], axis=1)


def kernel(x: np.ndarray, weights: np.ndarray, bias: np.ndarray) -> np.ndarray:
    from concourse.bass_utils import run_bass_kernel_spmd

    if "nc" not in _cache:
        _cache["nc"] = _build()
    nc = _cache["nc"]

    in_maps = make_in_maps(x, weights, bias)
    res = run_bass_kernel_spmd(nc, in_maps, core_ids=list(range(NCORES)))
    return gather(res.results)


# revision 22
# speedup vs baseline: 1.2158x; 1.2158x over previous
"""Trainium2 Bass kernel for channel-wise weighted reduction + capped relu.

Computes out[b, s] = capped_relu(sum_c x[b,c,s] * W[c,s] + bias[s]) for
x [64, 256, 4096] f32, W [256, 4096] f32, bias [4096] f32.

Sharding (v2): split the s axis (4096) across 8 NeuronCores — 512 columns
per core, ALL 64 batches. Each core reads x[:, :, s0:s0+512] (32 MiB), its
W slice [256, 512] (0.5 MiB) and bias slice. Compared to batch-parallel
sharding this cuts replicated-weight HBM traffic 8x (36 -> 32.6 MiB/core).

Per-core pipeline (v2):
  - 8 groups of 8 batches. Per group, 2 DMAs of [128c, 8b*512s] (2 MiB
    each, one per channel half) on the sync HWDGE ring.
  - DVE: y = x * Wrep elementwise, one [128, 4096] op per half (Wrep is
    the 0.5 KiB weight half replicated 8x along free at startup).
  - PE: channel reduction via ones[128,1] stationary matmul, one N=512
    chunk per (batch, half), accumulating h0+h1 in PSUM. Optional f32r
    (1 cyc/row vs 4 for f32); products are rounded to f32r by the DVE.
  - ACT drains psum rows to a [1, 4096] staging row; an accumulating
    SWDGE DMA packs it onto out_acc[g*8:(g+1)*8, :] which was preloaded
    with bias (bias add fused for free).
  - Epilogue on [64, 512]: capped relu via mask, single store.
"""

import os

import numpy as np

B, C, S = 64, 256, 4096
NCORES = 8
BPC = B // NCORES          # batches per core (v1 batch sharding)
SPC = S // NCORES          # s-columns per core (v2 s sharding)
H = C // 128               # 2 channel halves

_cache = {}


def _arch():
    return os.environ.get("K_ARCH", "v2")


def _build_nc_v2(fold=2, gfold=1, bufs=4, wbcast=True):
    """S-sharded kernel. Host stages x as [C, B*SPC] (c-major) so every
    DMA chunk is one contiguous run per partition (128 descriptors per
    2 MiB transfer instead of 1024 — HWDGE descriptor-gen was the v2
    bottleneck at 3.6 us per transfer).

    fold:  batches per group whose h0+h1 products are pre-added on DVE
    gfold: batches per group pre-added on GPSIMD (2x DVE cost, idle engine)
    wbcast: read W via a stride-0 broadcast AP instead of materializing
            an 8x-replicated copy (saves 3.5 MiB of SBUF->SBUF DMA).
    """
    import concourse.bacc as bacc
    import concourse.mybir as mybir
    from concourse.tile import TileContext

    f32 = mybir.dt.float32
    Alu = mybir.AluOpType

    NB = B // 8                # 8 groups of 8 batches
    GW = 8 * SPC               # free width of one group tile: 8*512 = 4096

    nc = bacc.Bacc(
        "TRN2",
        target_bir_lowering=False,
        debug=False,
        num_devices=NCORES,
    )

    x_d = nc.dram_tensor("x", [C, B * SPC], f32, kind="ExternalInput").ap()
    w_d = nc.dram_tensor("weights", [C, SPC], f32, kind="ExternalInput").ap()
    b_d = nc.dram_tensor("bias", [SPC], f32, kind="ExternalInput").ap()
    o_d = nc.dram_tensor("out", [B, SPC], f32, kind="ExternalOutput").ap()

    with TileContext(nc) as tc:
        with (
            tc.tile_pool(name="consts", bufs=1) as cpool,
            tc.tile_pool(name="xbuf", bufs=bufs) as xpool,
            tc.tile_pool(name="stg", bufs=2) as spool,
            tc.tile_pool(name="epi", bufs=1) as epool,
            tc.tile_pool(name="ps", bufs=1, space="PSUM") as ppool,
        ):
            WREP = 1 if wbcast else 8
            w_t = [cpool.tile([128, WREP * SPC], f32, name=f"w{h}") for h in range(H)]
            for h in range(H):
                nc.scalar.dma_start(w_t[h][:, 0:SPC], w_d[h * 128:(h + 1) * 128, :])
                if not wbcast:
                    for k in (1, 2, 4):
                        nc.scalar.activation(
                            w_t[h][:, k * SPC:2 * k * SPC], w_t[h][:, 0:k * SPC],
                            mybir.ActivationFunctionType.Copy,
                        )

            ones_t = cpool.tile([128, 1], f32, name="ones_t")
            nc.vector.memset(ones_t[:], 1.0)

            # out_acc preloaded with bias on every row; group sums are
            # packed on with accumulating SWDGE DMAs (fused bias add).
            out_acc = epool.tile([B, SPC], f32, name="out_acc")
            nc.gpsimd.dma_start(out_acc[0:1, :], b_d[None, :])
            k = 1
            while k < B:
                nc.gpsimd.dma_start(out_acc[k:2 * k, :], out_acc[0:k, :])
                k *= 2

            psum_big = ppool.tile([128, GW], f32, name="psum_big")
            rings = [nc.sync, nc.scalar]

            for g in range(NB):
                hb = g % 2              # psum bank half (free-dim half)
                rp = (g // 2) % 2       # psum row pair
                last = g == NB - 1
                xt = [
                    xpool.tile([128, GW], f32, name=f"x_h{h}", tag=f"x{h}", bufs=bufs)
                    for h in range(H)
                ]
                stg = spool.tile([1, GW], f32, name="stg", tag="stg")
                for h in range(H):
                    rings[int(os.environ.get("K_RING","0")) and h].dma_start(
                        xt[h][:, :],
                        x_d[h * 128:(h + 1) * 128, g * GW:(g + 1) * GW],
                    )
                # y = x * W (in place), one [128, 4096] op per half; the
                # last group runs in 2048-wide chunks to shorten the tail.
                nmul = 2 if last else 1
                for mq in range(nmul):
                    ms = slice(mq * GW // nmul, (mq + 1) * GW // nmul)
                    nb_lo = 8 // nmul * mq
                    nb_hi = 8 // nmul * (mq + 1)
                    for h in range(H):
                        if wbcast:
                            wop = (
                                w_t[h][:, 0:SPC]
                                .unsqueeze(1)
                                .broadcast_to([128, (nb_hi - nb_lo), SPC])
                            )
                        else:
                            wop = w_t[h][:, ms]
                        nc.vector.tensor_tensor(
                            xt[h][:, ms].rearrange(
                                "p (b s) -> p b s", b=nb_hi - nb_lo
                            ) if wbcast else xt[h][:, ms],
                            xt[h][:, ms].rearrange(
                                "p (b s) -> p b s", b=nb_hi - nb_lo
                            ) if wbcast else xt[h][:, ms],
                            wop,
                            Alu.mult,
                        )
                    # channel-sum via PE; batch b8 -> row 32*(2*rp + b8//4),
                    # free off 2048*hb + (b8%4)*512 (4 banks per group; bank
                    # half alternates per group so ACT drains don't collide
                    # with the next group's matmul writes). fold/gfold
                    # batches get h0+h1 pre-added on DVE/GPSIMD, halving
                    # their PE stream (f32 moving is 4 cyc/row).
                    for b8 in range(nb_lo, nb_hi):
                        row = 32 * (2 * rp + b8 // 4)
                        off = (GW // 2) * hb + (b8 % 4) * SPC
                        sl = slice(b8 * SPC, (b8 + 1) * SPC)
                        fold_eng = None
                        if b8 >= 8 - fold:
                            fold_eng = nc.vector
                        elif b8 < gfold:
                            fold_eng = nc.gpsimd
                        if fold_eng is not None:
                            fold_eng.tensor_tensor(
                                xt[0][:, sl], xt[0][:, sl], xt[1][:, sl], Alu.add
                            )
                        nh = 1 if fold_eng is not None else H
                        for h in range(nh):
                            nc.tensor.matmul(
                                psum_big[row:row + 1, off:off + SPC],
                                ones_t[:, 0:1],
                                xt[h][:, sl],
                                start=(h == 0),
                                stop=(h == nh - 1),
                                tile_position=(0, row),
                            )
                        if b8 % 4 == 3:
                            # this psum row is complete: drain it now so the
                            # ACT copy overlaps the remaining matmuls
                            half = b8 // 4
                            nc.scalar.activation(
                                stg[:, half * (GW // 2):(half + 1) * (GW // 2)],
                                psum_big[row:row + 1,
                                         (GW // 2) * hb:(GW // 2) * hb + GW // 2],
                                mybir.ActivationFunctionType.Copy,
                            )
                nc.gpsimd.dma_start(
                    out_acc[g * 8:(g + 1) * 8, :],
                    stg[:, :].rearrange("p (b s) -> p b s", b=8),
                    accum_op=Alu.add,
                )

            # capped relu on [64, 512], then single store
            msk = epool.tile([B, SPC], f32, name="msk")
            nc.vector.tensor_scalar(msk[:], out_acc[:, :], 0.0, 1.0, Alu.max, Alu.is_le)
            nc.vector.scalar_tensor_tensor(
                out_acc[:, :], out_acc[:, :], 0.0, msk[:], Alu.max, Alu.mult
            )
            nc.scalar.dma_start(o_d[:, :], out_acc[:, :])

    nc.compile()
    return nc


def _build_nc_v1(fold_batches=2, use_f32r=False):
    """Batch-sharded baseline (8 batches/core, W replicated)."""
    import concourse.bacc as bacc
    import concourse.bass as bass
    import concourse.mybir as mybir
    from concourse.tile import TileContext

    f32 = mybir.dt.float32
    Alu = mybir.AluOpType
    NJ = S // 512
    nc = bacc.Bacc(
        "TRN2",
        target_bir_lowering=False,
        debug=False,
        num_devices=NCORES,
    )

    x_d = nc.dram_tensor("x", [BPC, C, S], f32, kind="ExternalInput").ap()
    w_d = nc.dram_tensor("weights", [C, S], f32, kind="ExternalInput").ap()
    b_d = nc.dram_tensor("bias", [S], f32, kind="ExternalInput").ap()
    o_d = nc.dram_tensor("out", [BPC, S], f32, kind="ExternalOutput").ap()

    with TileContext(nc) as tc:
        NQ = 4
        QS = S // NQ
        with (
            tc.tile_pool(name="consts", bufs=1) as cpool,
            tc.tile_pool(name="xbuf", bufs=3) as xpool,
            tc.tile_pool(name="stg", bufs=2) as spool,
            tc.tile_pool(name="epi", bufs=1) as epool,
            tc.tile_pool(name="ps", bufs=1, space="PSUM") as ppool,
        ):
            w_t = cpool.tile([128, H * S], f32, name="w_t")

            ones_dt = mybir.dt.float32r if use_f32r else f32
            ones_t = cpool.tile([128, 1], ones_dt, name="ones_t")
            nc.vector.memset(ones_t[:], 1.0)

            psum_big = ppool.tile([128, S], f32, name="psum_big")
            out_acc = epool.tile([BPC, S], f32, name="out_acc")
            for bb in range(BPC):
                nc.scalar.dma_start(out_acc[bb:bb + 1, :], b_d[None, :])

            def chunk(base, h, q):
                return slice(base + h * S + q * QS, base + h * S + (q + 1) * QS)

            for b in range(BPC):
                hb = b % 2
                rp = (b // 2) % 2
                xh = [
                    xpool.tile([128, S], f32, name=f"x_h{h}", tag=f"x{h}", bufs=3)
                    for h in range(H)
                ]
                for h in range(H):
                    for dq in range(2):
                        lo, hi = dq * (S // 2), (dq + 1) * (S // 2)
                        if b == 0:
                            nc.sync.dma_start(
                                w_t[:, h * S + lo:h * S + hi],
                                w_d[h * 128:(h + 1) * 128, lo:hi],
                            )
                        nc.sync.dma_start(
                            xh[h][:, lo:hi],
                            x_d[b, h * 128:(h + 1) * 128, lo:hi],
                        )
                fold = b >= BPC - fold_batches
                nhalf = 1 if fold else H
                for q in range(NQ):
                    qs = slice(q * QS, (q + 1) * QS)
                    for h in range(H):
                        out_ap = xh[h][:, qs]
                        if use_f32r:
                            out_ap = out_ap.bitcast(mybir.dt.float32r)
                        nc.vector.tensor_tensor(
                            out_ap, xh[h][:, qs], w_t[:, chunk(0, h, q)],
                            Alu.mult,
                        )
                    if fold:
                        nc.vector.tensor_tensor(
                            xh[0][:, qs], xh[0][:, qs], xh[1][:, qs], Alu.add
                        )
                    for j in (2 * q, 2 * q + 1):
                        row = 32 * (2 * rp + j // 4)
                        off = (S // 2) * hb + (j % 4) * 512
                        for h in range(nhalf):
                            rhs = xh[h][:, j * 512:(j + 1) * 512]
                            lhsT = ones_t[:, 0:1]
                            if use_f32r:
                                rhs = rhs.bitcast(mybir.dt.float32r)
                            nc.tensor.matmul(
                                psum_big[row:row + 1, off:off + 512],
                                lhsT,
                                rhs,
                                start=(h == 0),
                                stop=(h == nhalf - 1),
                                tile_position=(0, row),
                            )
                stg = spool.tile([1, S], f32, name="stg", tag="stg")
                for half in range(2):
                    row = 32 * (2 * rp + half)
                    off = (S // 2) * hb
                    nc.scalar.activation(
                        stg[:, half * (S // 2):(half + 1) * (S // 2)],
                        psum_big[row:row + 1, off:off + S // 2],
                        mybir.ActivationFunctionType.Copy,
                    )
                nc.gpsimd.dma_start(
                    out_acc[b:b + 1, :], stg[:, :], accum_op=Alu.add
                )

            for s0 in (0, S // 2):
                sl = slice(s0, s0 + S // 2)
                msk = epool.tile([BPC, S // 2], f32, name="msk", tag="msk", bufs=1)
                nc.vector.tensor_scalar(msk[:], out_acc[:, sl], 0.0, 1.0, Alu.max, Alu.is_le)
                nc.vector.scalar_tensor_tensor(
                    out_acc[:, sl], out_acc[:, sl], 0.0, msk[:], Alu.max, Alu.mult
                )
                nc.scalar.dma_start(o_d[:, sl], out_acc[:, sl])

    nc.compile()
    return nc


def _build():
    if _arch() == "v1":
        return _build_nc_v1(
            fold_batches=int(os.environ.get("K_FOLD", "2")),
            use_f32r=bool(int(os.environ.get("K_F32R", "0"))),
        )
    return _build_nc_v2(
        fold=int(os.environ.get("K_FOLD", "2")),
        gfold=int(os.environ.get("K_GFOLD", "1")),
        bufs=int(os.environ.get("K_BUFS", "4")),
        wbcast=bool(int(os.environ.get("K_WBCAST", "1"))),
    )


def make_in_maps(x, weights, bias):
    x = np.ascontiguousarray(x, dtype=np.float32)
    weights = np.ascontiguousarray(weights, dtype=np.float32)
    bias = np.ascontiguousarray(bias, dtype=np.float32)
    if _arch() == "v1":
        return [
            {"x": x[i * BPC:(i + 1) * BPC], "weights": weights, "bias": bias}
            for i in range(NCORES)
        ]
    # v2: x staged c-major [C, B*SPC] so each partition's chunk data is one
    # contiguous DRAM run (128 DMA descriptors per 2 MiB transfer).
    return [
        {
            "x": np.ascontiguousarray(
                x[:, :, i * SPC:(i + 1) * SPC].transpose(1, 0, 2)
            ).reshape(C, B * SPC),
            "weights": np.ascontiguousarray(weights[:, i * SPC:(i + 1) * SPC]),
            "bias": np.ascontiguousarray(bias[i * SPC:(i + 1) * SPC]),
        }
        for i in range(NCORES)
    ]


def gather(results):
    if _arch() == "v1":
        return np.concatenate([results[i]["out"] for i in range(NCORES)], axis=0)
    return np.concatenate([results[i]["out"] for i in range(NCORES)], axis=1)


def kernel(x: np.ndarray, weights: np.ndarray, bias: np.ndarray) -> np.ndarray:
    from concourse.bass_utils import run_bass_kernel_spmd

    if "nc" not in _cache:
        _cache["nc"] = _build()
    nc = _cache["nc"]

    in_maps = make_in_maps(x, weights, bias)
    res = run_bass_kernel_spmd(nc, in_maps, core_ids=list(range(NCORES)))
    return gather(res.results)
